# revision 16
# baseline (speedup 1.0000x reference)
"""Trainium2 Bass kernel for quantum-augmented MultiHeadAttention.

Math: the per-head "quantum layer" is affine (pre-matmul, Givens rotations,
post-matmul, residual), so it folds into a 64x64 matrix applied to each
head's slice of the QKV projections.  The device kernel is then a plain
multi-head attention:
    q = query @ wq_eff.T ; k = key @ wk_eff.T ; v = value @ wv.T
    out = softmax(q k^T / 8) v @ wo.T
Sharding: 8 cores = (4 batches) x (2 head-groups of 8 heads).  Each core
computes its head-group's attention and a row-sharded partial of the output
projection; the two partials per batch are summed on the host.

Device layout (per core):
    QT/KT [512=8*64, S] head-dim-major (computed as W @ X^T), V [S, 8*65]
    with a ones column per head (gives the softmax denominator for free).
    Scores are computed transposed (S.T[sk, sq]) so every matmul operand is
    naturally laid out; exp on the scalar engine (the kernel bottleneck);
    O^T accumulated over sk tiles in PSUM; softmax normalization via
    gpsimd partition_broadcast of the reciprocal denominators.
Matmul operands are bf16 (the only full-rate PE dtype on this toolchain);
accumulation is fp32 in PSUM.  The projection weights are split into bf16
hi+lo pairs (split_w) to halve the projection rounding error -- the extra
matmuls hide under the ACT-engine exp bottleneck.  Phase order is arranged
so attention (ACT-bound) starts as early as possible: V projection first,
then K/Q projections head-block-major, attention head-block-major.
"""

import sys

sys.path.insert(0, "/opt/trn_rl_repo")

import numpy as np
import ml_dtypes

BF16 = ml_dtypes.bfloat16
EMBED = 1024
HEADS = 16
HD = 64
NQ = 6
HPC = 8  # heads per core
DC = HPC * HD  # 512 head dims per core
N_CORES = 8
S = 2048
SQ = 512  # sq chunk (PSUM bank width in fp32)
SPLIT_W = False  # hi/lo-split projection weights (better precision, PE-free)

_CACHE = {}


# ----------------------------------------------------------------- host math
def _rot_matrix(theta, phi):
    """64x64 matrix M with  res_out = res_in @ M  for the qubit rotations."""
    M = np.eye(HD, dtype=np.float64)
    idx = np.arange(HD)
    for i in range(NQ):
        c = np.cos(np.float64(theta[i]))
        s = np.sin(np.float64(theta[i]))
        cp = np.cos(np.float64(phi[i]))
        i0 = np.where(((idx >> i) & 1) == 0)[0]
        i1 = i0 + (1 << i)
        Mi = np.zeros((HD, HD), dtype=np.float64)
        Mi[i0, i0] = c
        Mi[i1, i0] = -s
        Mi[i0, i1] = s * cp
        Mi[i1, i1] = c * cp
        M = M @ Mi
    return M


def _quantum_fold(theta, phi, preW, preb, postW, postb):
    """quantum_layer(x) == x @ Weff + beff   (row-vector convention)."""
    M = _rot_matrix(theta, phi)
    core = preW.T.astype(np.float64) @ M @ postW.T.astype(np.float64)
    Weff = core + np.eye(HD)
    beff = preb.astype(np.float64) @ M @ postW.T.astype(np.float64) + postb
    return Weff, beff


def _fold_weights(wq, wk, q_fold, k_fold, scale_q):
    """Per-head fold of the quantum Weff into the projection weights."""
    Wq_eff = np.empty((EMBED, EMBED), dtype=np.float64)
    Wk_eff = np.empty((EMBED, EMBED), dtype=np.float64)
    for h in range(HEADS):
        sl = slice(h * HD, (h + 1) * HD)
        Wq_eff[:, sl] = wq[sl, :].astype(np.float64).T @ q_fold
        Wk_eff[:, sl] = wk[sl, :].astype(np.float64).T @ k_fold
    Wq_eff *= scale_q
    return Wq_eff.astype(np.float32), Wk_eff.astype(np.float32)


def _hi_lo(w):
    hi = w.astype(BF16)
    lo = (w - hi.astype(np.float32)).astype(BF16)
    return hi, lo


# ------------------------------------------------------------ device program
def _attn_hp_sqc(nc, mybir, seq_len, hp, sqc, qt, kt, vt, ot,
                 ptp, rcp, stp, opp, filler=None, norm_group=None):
    """Attention for head-pair hp (heads 2hp, 2hp+1), one sq chunk.

    filler: optional callable emitting a couple of independent PE
    instructions per call (next round's projection / out-projection
    matmuls) -- fills the PE slots where O-matmuls wait on exp.

    norm_group: dict with keys den (staging tile [8, SQ]) and rows
    (list).  The unnormalized O^T chunk is staged to SBUF (osb) and its
    denominator row gathered into den[row]; the division is deferred to
    _flush_norm (batched reciprocal -- a [1, SQ] DVE reciprocal costs as
    much as a [128, SQ] one, so batching rows is ~n x cheaper)."""
    f32 = mybir.dt.float32
    bf16 = mybir.dt.bfloat16
    FN = mybir.ActivationFunctionType
    nskt = seq_len // 128

    if True:
        o_ps = [opp.tile([128, SQ], f32, name=f"o_ps{j}", tag="o_ps")
                for j in range(2)]

        def emit_scores(skt):
            # the j=0/j=1 pair sits at PE row groups 0-63/64-127 (auto
            # tile_position), so the two K=64 matmuls overlap in the array
            st = stp.tile([128, 2 * SQ], f32, name="st", tag="st")
            for j in range(2):
                nc.tensor.matmul(
                    st[:, j * SQ:(j + 1) * SQ],
                    kt[hp][j * HD:(j + 1) * HD, skt * 128:(skt + 1) * 128],
                    qt[hp][j * HD:(j + 1) * HD, sqc * SQ:(sqc + 1) * SQ],
                    start=True, stop=True)
            return st

        # Software pipeline: scores(k+1) is emitted BEFORE the O matmuls of
        # slot k.  The PE queue is in-order, and O(k) blocks on exp(k) -- with
        # the naive order scores(k+1) would sit behind that stall and starve
        # the ACT engine of its next exp input.
        st = emit_scores(0)
        for skt in range(nskt):
            pt = ptp.tile([128, 2 * SQ], bf16, name="pt", tag="pt")
            nc.scalar.activation(pt[:], st[:], FN.Exp)
            if skt + 1 < nskt:
                st = emit_scores(skt + 1)
            if filler is not None:
                filler()
            for j in range(2):
                h = 2 * hp + j
                nc.tensor.matmul(
                    o_ps[j][0:HD + 1, :],
                    vt[skt][:, h * (HD + 1):(h + 1) * (HD + 1)],
                    pt[:, j * SQ:(j + 1) * SQ],
                    start=(skt == 0), stop=(skt == nskt - 1))
        for j in range(2):
            # one fast PSUM read frees the bank for the next pair
            osb = rcp.tile([HD + 1, SQ], f32, name="osb", tag="osb", bufs=5)
            nc.vector.tensor_copy(osb[:], o_ps[j][0:HD + 1, :])
            row = len(norm_group["rows"])
            # cross-partition gather of the denominator row (tiny SBUF DMA)
            nc.sync.dma_start(norm_group["den"][row:row + 1, :],
                              osb[HD:HD + 1, :])
            norm_group["rows"].append((hp, sqc, j, osb))


def _flush_norm(nc, mybir, rcp, ot, norm_group):
    """Batched softmax normalization: one reciprocal for all staged rows,
    then per-row Pool broadcast + DVE multiply into ot."""
    f32 = mybir.dt.float32
    rows = norm_group["rows"]
    if not rows:
        return
    n = len(rows)
    # one batched reciprocal for the whole group -- a [1, SQ] DVE reciprocal
    # costs the same as an [n, SQ] one (free-size-driven), so this is ~n x
    # cheaper than per-row reciprocals
    denr = rcp.tile([8, SQ], f32, name="denr", tag="denr")
    nc.vector.reciprocal(denr[0:n, :], norm_group["den"][0:n, :])
    for i, (hp, sqc, j, osb) in enumerate(rows):
        # scatter row i back to partition 0 (tiny DMA) so the Pool-engine
        # broadcast reads the same base-0 [1, SQ] shape the HW ucode expects
        rcb = rcp.tile([1, SQ], f32, name="rcb", tag="rcb", bufs=2)
        nc.sync.dma_start(rcb[0:1, :], denr[i:i + 1, :])
        bcs = rcp.tile([HD, SQ], f32, name="bcs", tag="bcs", bufs=3)
        nc.gpsimd.partition_broadcast(bcs[:], rcb[0:1, :])
        nc.vector.tensor_mul(
            ot[hp][j * HD:(j + 1) * HD, sqc * SQ:(sqc + 1) * SQ],
            osb[0:HD, :], bcs[:])
    norm_group["rows"] = []
    norm_group["den"] = rcp.tile([8, SQ], f32, name="den", tag="den")


def build_program(seq_len=S, loop_n=None, split_w=SPLIT_W, phases="pao"):
    """Build the per-core Bass program.  Returns a compiled Bacc.

    The body is emitted in interleaved rounds -- V projection first, then
    per head-block m: K-proj(m), Q-proj(m), attention(head-pair m) -- so the
    ACT engine (exp, the bottleneck) starts ~30us in and the projection
    matmuls hide under it.

    loop_n: wrap the body in a For_i hardware loop (timing).  phases:
    subset of "pao" for phase-bisection timing.
    """
    import concourse.tile as tile
    from concourse import bacc, mybir
    from contextlib import ExitStack, nullcontext

    f32 = mybir.dt.float32
    bf16 = mybir.dt.bfloat16
    nsqc = seq_len // SQ
    nskt = seq_len // 128
    nwq = 2 if split_w in (True, "both", "q") else 1
    nwk = 2 if split_w in (True, "both") else 1

    nc = bacc.Bacc("TRN2", target_bir_lowering=False, debug=False,
                   num_devices=N_CORES)

    xq = nc.dram_tensor("xq_t", [EMBED, seq_len], bf16, kind="ExternalInput").ap()
    xk = nc.dram_tensor("xk_t", [EMBED, seq_len], bf16, kind="ExternalInput").ap()
    xv = nc.dram_tensor("xv_t", [EMBED, seq_len], bf16, kind="ExternalInput").ap()
    wqd = nc.dram_tensor("wq_t", [nwq, EMBED, DC], bf16, kind="ExternalInput").ap()
    wkd = nc.dram_tensor("wk_t", [nwk, EMBED, DC], bf16, kind="ExternalInput").ap()
    wvd = nc.dram_tensor("wv_t", [1, EMBED, DC], bf16, kind="ExternalInput").ap()
    wod = nc.dram_tensor("wo_t", [DC, EMBED], bf16, kind="ExternalInput").ap()
    onesd = nc.dram_tensor("ones_d", [128, HPC], bf16, kind="ExternalInput").ap()
    outd = nc.dram_tensor("out", [seq_len, EMBED], f32, kind="ExternalOutput").ap()

    with tile.TileContext(nc) as tc, ExitStack() as top:
        qkv = top.enter_context(tc.tile_pool(name="qkv", bufs=1))
        qt = [qkv.tile([128, seq_len], bf16, name=f"qt{m}", tag=f"qt{m}")
              for m in range(4)]
        kt = [qkv.tile([128, seq_len], bf16, name=f"kt{m}", tag=f"kt{m}")
              for m in range(4)]
        vt = [qkv.tile([128, HPC * (HD + 1)], bf16, name=f"vt{i}", tag=f"vt{i}")
              for i in range(nskt)]
        otp = top.enter_context(tc.tile_pool(name="ot", bufs=1))
        ot = [otp.tile([128, seq_len], bf16, name=f"ot{m}", tag=f"ot{m}")
              for m in range(4)]

        loop = tc.For_i(0, loop_n, 1) if loop_n else nullcontext()
        with loop:
            # V ones columns (softmax denominator accumulators)
            for i in range(nskt):
                dst = vt[i][:].rearrange("p (h c) -> p h c", h=HPC)[:, :, HD]
                nc.sync.dma_start(dst, onesd[:, :])

            with tc.tile_pool(name="wp", bufs=1) as wp, \
                 tc.tile_pool(name="xr", bufs=1) as xr, \
                 tc.tile_pool(name="pt", bufs=3) as ptp, \
                 tc.tile_pool(name="rc", bufs=2) as rcp, \
                 tc.tile_pool(name="pp", bufs=2, space="PSUM") as pp, \
                 tc.tile_pool(name="stp", bufs=2, space="PSUM") as stp, \
                 tc.tile_pool(name="op", bufs=2, space="PSUM") as opp:

                if "p" in phases:
                    # weights first in consumption order: wv feeds the very
                    # first matmuls
                    wsb = {}
                    for name, dram, parts in (("v", wvd, 1), ("k", wkd, nwk),
                                              ("q", wqd, nwq)):
                        wsb[name] = [wp.tile([128, DC], bf16,
                                             name=f"w_{name}{k}",
                                             tag=f"w_{name}{k}")
                                     for k in range(8 * parts)]
                    for k in range(8):
                        nc.sync.dma_start(wsb["v"][k][:],
                                          wvd[0, k * 128:(k + 1) * 128, :])
                    xkr = [xr.tile([128, seq_len], bf16, name=f"xk{k}",
                                   tag=f"xk{k}") for k in range(8)]
                    xqr = [xr.tile([128, seq_len], bf16, name=f"xq{k}",
                                   tag=f"xq{k}") for k in range(8)]

                    # --- V projection (streamed X^T chunks) ---
                    with tc.tile_pool(name="xpv", bufs=2) as xpv:
                        for sqc in range(nsqc):
                            xtiles = []
                            for k in range(8):
                                t = xpv.tile([128, SQ], bf16, name=f"x{k}",
                                             tag=f"x{k}")
                                nc.sync.dma_start(
                                    t[:], xv[k * 128:(k + 1) * 128,
                                             sqc * SQ:(sqc + 1) * SQ])
                                xtiles.append(t)
                            for st4 in range(SQ // 128):
                                i = sqc * (SQ // 128) + st4
                                ps = pp.tile([128, DC], f32, name="ps", tag="ps")
                                for k in range(8):
                                    nc.tensor.matmul(
                                        ps[:],
                                        xtiles[k][:, st4 * 128:(st4 + 1) * 128],
                                        wsb["v"][k][:],
                                        start=(k == 0), stop=(k == 7))
                                src = ps[:].rearrange("p (h c) -> p h c", h=HPC)
                                dst = vt[i][:].rearrange(
                                    "p (h c) -> p h c", h=HPC)[:, :, 0:HD]
                                nc.vector.tensor_copy(dst, src)
                    # k/q weights + resident X^T (DMA overlaps V compute)
                    for name, dram in (("k", wkd), ("q", wqd)):
                        nwx = nwk if name == "k" else nwq
                        for p in range(nwx):
                            for k in range(8):
                                nc.sync.dma_start(
                                    wsb[name][p * 8 + k][:],
                                    dram[p, k * 128:(k + 1) * 128, :])
                    for k in range(8):
                        nc.sync.dma_start(xkr[k][:], xk[k * 128:(k + 1) * 128, :])
                    for k in range(8):
                        nc.sync.dma_start(xqr[k][:], xq[k * 128:(k + 1) * 128, :])
                else:
                    for m in range(4):
                        nc.sync.dma_start(qt[m][:], xq[m * 128:(m + 1) * 128, :])
                        nc.sync.dma_start(kt[m][:], xk[m * 128:(m + 1) * 128, :])
                    for i in range(nskt):
                        dst = vt[i][:].rearrange(
                            "p (h c) -> p h c", h=HPC)[:, :, 0:HD]
                        off = (i % (seq_len // 512)) * 512
                        nc.sync.dma_start(
                            dst, xv[0:128, off:off + 512].rearrange(
                                "p (h c) -> p h c", h=HPC))

                # --- software-pipelined rounds ---------------------------
                # Round m runs attention for head-pair m.  The projection
                # matmuls for round m+1 (and, in the last round, the output
                # projection) are emitted two-at-a-time inside the skt loop,
                # filling the PE slots that would otherwise stall on exp.
                def proj_round_gen(m):
                    if "p" not in phases:
                        return
                    for name, xres, dstb in (("k", xkr, kt), ("q", xqr, qt)):
                        nwx = nwk if name == "k" else nwq
                        for sq0 in range(0, nsqc, 2):
                            cs = [c for c in (sq0, sq0 + 1) if c < nsqc]
                            pss = [pp.tile([128, SQ], f32, name="ps", tag="ps")
                                   for _ in cs]
                            for k in range(8):
                                for p in range(nwx):
                                    for ci, sqc in enumerate(cs):
                                        nc.tensor.matmul(
                                            pss[ci][:],
                                            wsb[name][p * 8 + k][
                                                :, m * 128:(m + 1) * 128],
                                            xres[k][:, sqc * SQ:(sqc + 1) * SQ],
                                            start=(k == 0 and p == 0),
                                            stop=(k == 7 and p == nwx - 1))
                                    yield
                            for ci, sqc in enumerate(cs):
                                nc.vector.tensor_copy(
                                    dstb[m][:, sqc * SQ:(sqc + 1) * SQ],
                                    pss[ci][:])

                with tc.tile_pool(name="wo", bufs=1) as wop, \
                     tc.tile_pool(name="ob", bufs=2) as obp:
                    wo_sb = [wop.tile([128, EMBED], bf16, name=f"wo{k}",
                                      tag=f"wo{k}") for k in range(4)]
                    if "o" in phases:
                        for k in range(4):
                            nc.sync.dma_start(wo_sb[k][:],
                                              wod[k * 128:(k + 1) * 128, :])

                    def outproj_gen(mts):
                        if "o" not in phases:
                            return
                        for mt in mts:
                            ob = obp.tile([128, EMBED], f32, name="ob",
                                          tag="ob")
                            pss = [pp.tile([128, SQ], f32, name="ps", tag="ps")
                                   for _ in range(2)]
                            for kb in range(4):
                                for nch in range(2):
                                    nc.tensor.matmul(
                                        pss[nch][:],
                                        ot[kb][:, mt * 128:(mt + 1) * 128],
                                        wo_sb[kb][:, nch * 512:(nch + 1) * 512],
                                        start=(kb == 0), stop=(kb == 3))
                                yield
                            for nch in range(2):
                                nc.vector.tensor_copy(
                                    ob[:, nch * 512:(nch + 1) * 512],
                                    pss[nch][:])
                            nc.sync.dma_start(
                                outd[mt * 128:(mt + 1) * 128, :], ob[:])

                    # round-0 projections up front
                    for _ in proj_round_gen(0):
                        pass

                    norm_group = {"den": rcp.tile([8, SQ], f32, name="den",
                                                  tag="den"),
                                  "rows": []}
                    for m in range(4):
                        # Rounds 0-2: filler = next round's projections;
                        # softmax norm flushed per 2 sq chunks (batched
                        # reciprocal).  Round 3: per-chunk norm flush; the
                        # out-projection chases it one chunk behind at 2
                        # filler matmul groups per slot.
                        gen = proj_round_gen(m + 1) if m < 3 else None
                        for sqc in range(nsqc):
                            if m == 3:
                                # out-proj of chunk sqc-1 (ot rows flushed)
                                gen = outproj_gen(
                                    range((sqc - 1) * 4, sqc * 4)) \
                                    if sqc > 0 and "a" in phases else None
                            if gen is None:
                                filler = lambda: None
                            elif m == 3:
                                filler = (lambda g=gen: (next(g, None),
                                                         next(g, None)))
                            else:
                                filler = (lambda g=gen: next(g, None))
                            if "a" in phases:
                                _attn_hp_sqc(nc, mybir, seq_len, m, sqc,
                                             qt, kt, vt, ot,
                                             ptp, rcp, stp, opp, filler,
                                             norm_group)
                                if m == 3 or sqc % 2 == 1:
                                    _flush_norm(nc, mybir, rcp, ot,
                                                norm_group)
                            else:
                                for _ in range(nskt):
                                    filler()
                            if m == 3 and gen is not None:
                                for _ in gen:  # drain this chunk's out-proj
                                    pass
                        if m < 3 and gen is not None:
                            for _ in gen:  # drain leftover filler work
                                pass
                        if "a" not in phases:
                            nc.sync.dma_start(ot[m][:],
                                              xq[m * 128:(m + 1) * 128, :])
                    # tail: last sq chunk's out-projection
                    for _ in (outproj_gen(range(12, 16)) if "a" in phases
                              else outproj_gen(range(16))):
                        pass

    nc.compile()
    return nc


# ----------------------------------------------------------------- interface
def _prepare(inputs, seq_len, split_w=SPLIT_W):
    """Host-side fold + shard.  Returns (in_maps, bo)."""
    q_fold, q_beff = _quantum_fold(inputs["q_theta"], inputs["q_phi"],
                                   inputs["q_preW"], inputs["q_preb"],
                                   inputs["q_postW"], inputs["q_postb"])
    k_fold, k_beff = _quantum_fold(inputs["k_theta"], inputs["k_phi"],
                                   inputs["k_preW"], inputs["k_preb"],
                                   inputs["k_postW"], inputs["k_postb"])
    for b in (inputs["bq"], inputs["bk"], inputs["bv"], q_beff, k_beff):
        assert np.abs(np.asarray(b, dtype=np.float64)).max() == 0.0, \
            "nonzero bias path not implemented"

    scale_q = 1.0 / np.sqrt(np.float32(HD))
    wq_eff, wk_eff = _fold_weights(inputs["wq"], inputs["wk"],
                                   q_fold, k_fold, scale_q)
    wv_t = np.ascontiguousarray(inputs["wv"].T).astype(np.float32)
    wo = inputs["wo"]

    def wparts(w, split):  # [E, DC_slice] fp32 -> [parts, E, DC] bf16
        if split:
            hi, lo = _hi_lo(w)
            return np.stack([hi, lo])
        return w.astype(BF16)[None]

    B = inputs["query"].shape[0]
    xq_t = np.ascontiguousarray(inputs["query"].transpose(0, 2, 1)).astype(BF16)
    xk_t = np.ascontiguousarray(inputs["key"].transpose(0, 2, 1)).astype(BF16)
    xv_t = np.ascontiguousarray(inputs["value"].transpose(0, 2, 1)).astype(BF16)

    in_maps = []
    for c in range(N_CORES):
        b, hg = divmod(c, 2)
        b = b % B
        sl = slice(hg * DC, (hg + 1) * DC)
        in_maps.append({
            "xq_t": xq_t[b, :, :seq_len],
            "xk_t": xk_t[b, :, :seq_len],
            "xv_t": xv_t[b, :, :seq_len],
            "wq_t": wparts(wq_eff[:, sl], split_w in (True, "both", "q")),
            "wk_t": wparts(wk_eff[:, sl], split_w in (True, "both")),
            "wv_t": wv_t[:, sl].astype(BF16)[None],
            "wo_t": np.ascontiguousarray(wo[:, sl].T).astype(BF16),
            "ones_d": np.ones((128, HPC), dtype=BF16),
        })
    return in_maps, inputs["bo"]


def _install_hook_tracer():
    """Surface compile errors that the PJRT layer otherwise swallows."""
    if _CACHE.get("hook"):
        return
    _CACHE["hook"] = True
    try:
        from concourse import bass2jax
        bass2jax.install_neuronx_cc_hook()
        import libneuronxla
        orig = libneuronxla.neuronx_cc

        def wrapped(*args, **kwargs):
            try:
                return orig(*args, **kwargs)
            except Exception:
                import traceback
                traceback.print_exc()
                raise
        libneuronxla.neuronx_cc = wrapped
    except Exception:
        pass


def kernel(**inputs):
    from concourse.bass_utils import run_bass_kernel_spmd

    _install_hook_tracer()
    if "prog" not in _CACHE:
        _CACHE["prog"] = build_program(S)
    nc = _CACHE["prog"]

    in_maps, bo = _prepare(inputs, S)
    res = run_bass_kernel_spmd(nc, in_maps, core_ids=list(range(N_CORES)))

    B = inputs["query"].shape[0]
    out = np.empty((B, S, EMBED), dtype=np.float32)
    for b in range(B):
        out[b] = res.results[2 * b]["out"] + res.results[2 * b + 1]["out"]
    out += np.asarray(inputs["bo"]).reshape(1, 1, EMBED).astype(np.float32)
    return out



# revision 26
# speedup vs baseline: 1.0454x; 1.0454x over previous
"""Trainium2 Bass kernel for quantum-augmented MultiHeadAttention.

Math: the per-head "quantum layer" is affine (pre-matmul, Givens rotations,
post-matmul, residual), so it folds into a 64x64 matrix applied to each
head's slice of the QKV projections.  The device kernel is then a plain
multi-head attention:
    q = query @ wq_eff.T ; k = key @ wk_eff.T ; v = value @ wv.T
    out = softmax(q k^T / 8) v @ wo.T
Sharding: 8 cores = (4 batches) x (2 head-groups of 8 heads).  Each core
computes its head-group's attention and a row-sharded partial of the output
projection; the two partials per batch are summed on the host.

Device layout (per core):
    QT/KT [512=8*64, S] head-dim-major (computed as W @ X^T), V [S, 8*65]
    with a ones column per head (gives the softmax denominator for free).
    Scores are computed transposed (S.T[sk, sq]) so every matmul operand is
    naturally laid out; exp on the scalar engine (the kernel bottleneck);
    O^T accumulated over sk tiles in PSUM; softmax normalization via
    gpsimd partition_broadcast of the reciprocal denominators.
Matmul operands are bf16 (the only full-rate PE dtype on this toolchain);
accumulation is fp32 in PSUM.  The projection weights are split into bf16
hi+lo pairs (split_w) to halve the projection rounding error -- the extra
matmuls hide under the ACT-engine exp bottleneck.  Phase order is arranged
so attention (ACT-bound) starts as early as possible: V projection first,
then K/Q projections head-block-major, attention head-block-major.
"""

import sys

sys.path.insert(0, "/opt/trn_rl_repo")

import numpy as np
import ml_dtypes

BF16 = ml_dtypes.bfloat16
EMBED = 1024
HEADS = 16
HD = 64
NQ = 6
HPC = 8  # heads per core
DC = HPC * HD  # 512 head dims per core
N_CORES = 8
S = 2048
SQ = 512  # sq chunk (PSUM bank width in fp32)
SPLIT_W = False  # hi/lo-split projection weights (better precision, PE-free)

_CACHE = {}


# ----------------------------------------------------------------- host math
def _rot_matrix(theta, phi):
    """64x64 matrix M with  res_out = res_in @ M  for the qubit rotations."""
    M = np.eye(HD, dtype=np.float64)
    idx = np.arange(HD)
    for i in range(NQ):
        c = np.cos(np.float64(theta[i]))
        s = np.sin(np.float64(theta[i]))
        cp = np.cos(np.float64(phi[i]))
        i0 = np.where(((idx >> i) & 1) == 0)[0]
        i1 = i0 + (1 << i)
        Mi = np.zeros((HD, HD), dtype=np.float64)
        Mi[i0, i0] = c
        Mi[i1, i0] = -s
        Mi[i0, i1] = s * cp
        Mi[i1, i1] = c * cp
        M = M @ Mi
    return M


def _quantum_fold(theta, phi, preW, preb, postW, postb):
    """quantum_layer(x) == x @ Weff + beff   (row-vector convention)."""
    M = _rot_matrix(theta, phi)
    core = preW.T.astype(np.float64) @ M @ postW.T.astype(np.float64)
    Weff = core + np.eye(HD)
    beff = preb.astype(np.float64) @ M @ postW.T.astype(np.float64) + postb
    return Weff, beff


def _fold_weights(wq, wk, q_fold, k_fold, scale_q):
    """Per-head fold of the quantum Weff into the projection weights."""
    Wq_eff = np.empty((EMBED, EMBED), dtype=np.float64)
    Wk_eff = np.empty((EMBED, EMBED), dtype=np.float64)
    for h in range(HEADS):
        sl = slice(h * HD, (h + 1) * HD)
        Wq_eff[:, sl] = wq[sl, :].astype(np.float64).T @ q_fold
        Wk_eff[:, sl] = wk[sl, :].astype(np.float64).T @ k_fold
    Wq_eff *= scale_q
    return Wq_eff.astype(np.float32), Wk_eff.astype(np.float32)


def _hi_lo(w):
    hi = w.astype(BF16)
    lo = (w - hi.astype(np.float32)).astype(BF16)
    return hi, lo


# ------------------------------------------------------------ device program
def _attn_hp_sqc(nc, mybir, seq_len, hp, sqc, qt, kt, vt, ot,
                 ptp, rcp, stp, opp, filler=None, norm_group=None):
    """Attention for head-pair hp (heads 2hp, 2hp+1), one sq chunk.

    filler: optional callable emitting a couple of independent PE
    instructions per call (next round's projection / out-projection
    matmuls) -- fills the PE slots where O-matmuls wait on exp.

    norm_group: dict with keys den (staging tile [8, SQ]) and rows
    (list).  The unnormalized O^T chunk is staged to SBUF (osb) and its
    denominator row gathered into den[row]; the division is deferred to
    _flush_norm (batched reciprocal -- a [1, SQ] DVE reciprocal costs as
    much as a [128, SQ] one, so batching rows is ~n x cheaper)."""
    f32 = mybir.dt.float32
    bf16 = mybir.dt.bfloat16
    FN = mybir.ActivationFunctionType
    nskt = seq_len // 128

    if True:
        o_ps = [opp.tile([128, SQ], f32, name=f"o_ps{j}", tag="o_ps")
                for j in range(2)]

        def emit_scores(skt):
            # the j=0/j=1 pair sits at PE row groups 0-63/64-127 (auto
            # tile_position), so the two K=64 matmuls overlap in the array
            st = stp.tile([128, 2 * SQ], f32, name="st", tag="st")
            for j in range(2):
                nc.tensor.matmul(
                    st[:, j * SQ:(j + 1) * SQ],
                    kt[hp][j * HD:(j + 1) * HD, skt * 128:(skt + 1) * 128],
                    qt[hp][j * HD:(j + 1) * HD, sqc * SQ:(sqc + 1) * SQ],
                    start=True, stop=True)
            return st

        # Software pipeline: scores(k+1) is emitted BEFORE the O matmuls of
        # slot k.  The PE queue is in-order, and O(k) blocks on exp(k) -- with
        # the naive order scores(k+1) would sit behind that stall and starve
        # the ACT engine of its next exp input.
        st = emit_scores(0)
        for skt in range(nskt):
            pt = ptp.tile([128, 2 * SQ], bf16, name="pt", tag="pt")
            nc.scalar.activation(pt[:], st[:], FN.Exp)
            if skt + 1 < nskt:
                st = emit_scores(skt + 1)
            if filler is not None:
                filler()
            for j in range(2):
                h = 2 * hp + j
                nc.tensor.matmul(
                    o_ps[j][0:HD + 1, :],
                    vt[skt][:, h * (HD + 1):(h + 1) * (HD + 1)],
                    pt[:, j * SQ:(j + 1) * SQ],
                    start=(skt == 0), stop=(skt == nskt - 1))
        for j in range(2):
            # one fast PSUM read frees the bank for the next pair
            osb = rcp.tile([HD + 1, SQ], f32, name="osb", tag="osb", bufs=5)
            nc.vector.tensor_copy(osb[:], o_ps[j][0:HD + 1, :])
            row = len(norm_group["rows"])
            # cross-partition gather of the denominator row (tiny SBUF DMA)
            nc.sync.dma_start(norm_group["den"][row:row + 1, :],
                              osb[HD:HD + 1, :])
            norm_group["rows"].append((hp, sqc, j, osb))


def _flush_norm(nc, mybir, rcp, ot, norm_group):
    """Batched softmax normalization: one reciprocal for all staged rows,
    then per-row Pool broadcast + DVE multiply into ot."""
    f32 = mybir.dt.float32
    rows = norm_group["rows"]
    if not rows:
        return
    n = len(rows)
    # one batched reciprocal for the whole group -- a [1, SQ] DVE reciprocal
    # costs the same as an [n, SQ] one (free-size-driven), so this is ~n x
    # cheaper than per-row reciprocals
    denr = rcp.tile([8, SQ], f32, name="denr", tag="denr")
    nc.vector.reciprocal(denr[0:n, :], norm_group["den"][0:n, :])
    for i, (hp, sqc, j, osb) in enumerate(rows):
        # scatter row i back to partition 0 (tiny DMA) so the Pool-engine
        # broadcast reads the same base-0 [1, SQ] shape the HW ucode expects
        rcb = rcp.tile([1, SQ], f32, name="rcb", tag="rcb", bufs=2)
        nc.sync.dma_start(rcb[0:1, :], denr[i:i + 1, :])
        bcs = rcp.tile([HD, SQ], f32, name="bcs", tag="bcs", bufs=3)
        nc.gpsimd.partition_broadcast(bcs[:], rcb[0:1, :])
        nc.vector.tensor_mul(
            ot[hp][j * HD:(j + 1) * HD, sqc * SQ:(sqc + 1) * SQ],
            osb[0:HD, :], bcs[:])
    norm_group["rows"] = []
    norm_group["den"] = rcp.tile([8, SQ], f32, name="den", tag="den")


def build_program(seq_len=S, loop_n=None, split_w=SPLIT_W, phases="pao"):
    """Build the per-core Bass program.  Returns a compiled Bacc.

    The body is emitted in interleaved rounds -- V projection first, then
    per head-block m: K-proj(m), Q-proj(m), attention(head-pair m) -- so the
    ACT engine (exp, the bottleneck) starts ~30us in and the projection
    matmuls hide under it.

    loop_n: wrap the body in a For_i hardware loop (timing).  phases:
    subset of "pao" for phase-bisection timing.
    """
    import concourse.tile as tile
    from concourse import bacc, mybir
    from contextlib import ExitStack, nullcontext

    f32 = mybir.dt.float32
    bf16 = mybir.dt.bfloat16
    nsqc = seq_len // SQ
    nskt = seq_len // 128
    nwq = 2 if split_w in (True, "both", "q") else 1
    nwk = 2 if split_w in (True, "both") else 1

    nc = bacc.Bacc("TRN2", target_bir_lowering=False, debug=False,
                   num_devices=N_CORES)

    # Pre-tiled DRAM layouts (host-prepared): contiguous per partition so each
    # tensor loads in ONE DMA of 128 single-run descriptors.  HWDGE descriptor
    # generation is ~5 ns/descriptor and serializes DMAs; the naive per-tile
    # layout cost ~18 us of desc-gen before the first matmul.
    # xq/xk: [p, k*S + s] = X^T[128k+p, s];  xv: [p, sqc*4096 + k*512 + j]
    xq = nc.dram_tensor("xq_t", [128, 8 * seq_len], bf16,
                        kind="ExternalInput").ap()
    xk = nc.dram_tensor("xk_t", [128, 8 * seq_len], bf16,
                        kind="ExternalInput").ap()
    xv = nc.dram_tensor("xv_t", [128, 8 * seq_len], bf16,
                        kind="ExternalInput").ap()
    # weights: [p, k*DC + j] = W[128k+p, j];  wo: [p, kb*EMBED + e]
    wqd = nc.dram_tensor("wq_t", [128, 8 * DC], bf16, kind="ExternalInput").ap()
    wkd = nc.dram_tensor("wk_t", [128, 8 * DC], bf16, kind="ExternalInput").ap()
    wvd = nc.dram_tensor("wv_t", [128, 8 * DC], bf16, kind="ExternalInput").ap()
    wod = nc.dram_tensor("wo_t", [128, 4 * EMBED], bf16,
                         kind="ExternalInput").ap()
    outd = nc.dram_tensor("out", [seq_len, EMBED], f32, kind="ExternalOutput").ap()
    assert not split_w, "pre-tiled layouts assume unsplit weights"

    with tile.TileContext(nc) as tc, ExitStack() as top:
        qkv = top.enter_context(tc.tile_pool(name="qkv", bufs=1))
        qt = [qkv.tile([128, seq_len], bf16, name=f"qt{m}", tag=f"qt{m}")
              for m in range(4)]
        kt = [qkv.tile([128, seq_len], bf16, name=f"kt{m}", tag=f"kt{m}")
              for m in range(4)]
        vt = [qkv.tile([128, HPC * (HD + 1)], bf16, name=f"vt{i}", tag=f"vt{i}")
              for i in range(nskt)]
        otp = top.enter_context(tc.tile_pool(name="ot", bufs=1))
        ot = [otp.tile([128, seq_len], bf16, name=f"ot{m}", tag=f"ot{m}")
              for m in range(4)]

        loop = tc.For_i(0, loop_n, 1) if loop_n else nullcontext()
        with loop:
            # V ones columns (softmax denominator accumulators) -- DVE memset,
            # not DMA: 16 tiny DMAs would serialize ~10us of HWDGE desc-gen
            for i in range(nskt):
                dst = vt[i][:].rearrange("p (h c) -> p h c", h=HPC)[:, :, HD]
                nc.vector.memset(dst, 1.0)

            with tc.tile_pool(name="wp", bufs=1) as wp, \
                 tc.tile_pool(name="xr", bufs=1) as xr, \
                 tc.tile_pool(name="pt", bufs=3) as ptp, \
                 tc.tile_pool(name="rc", bufs=2) as rcp, \
                 tc.tile_pool(name="pp", bufs=2, space="PSUM") as pp, \
                 tc.tile_pool(name="stp", bufs=2, space="PSUM") as stp, \
                 tc.tile_pool(name="op", bufs=2, space="PSUM") as opp:

                if "p" in phases:
                    # weights first in consumption order: wv feeds the very
                    # first matmuls.  One DMA per tensor (pre-tiled DRAM).
                    wsb = {}
                    for name in ("v", "k", "q"):
                        wsb[name] = wp.tile([128, 8 * DC], bf16,
                                            name=f"w_{name}", tag=f"w_{name}")
                    nc.sync.dma_start(wsb["v"][:], wvd[:, :])
                    xkr = xr.tile([128, 8 * seq_len], bf16, name="xkr",
                                  tag="xkr")
                    xqr = xr.tile([128, 8 * seq_len], bf16, name="xqr",
                                  tag="xqr")

                    # --- V projection (streamed X^T chunks) ---
                    # DMA queue order paces consumption: wv, xv0, xv1, xkr,
                    # wk, xv2, xv3, xqr, wq -- K-proj inputs land under
                    # V-proj compute, Q-proj inputs under K-proj.
                    with tc.tile_pool(name="xpv", bufs=2) as xpv:
                        xts = {}

                        def xv_dma(sqc):
                            t = xpv.tile([128, 8 * SQ], bf16, name="xt",
                                         tag="xt")
                            nc.sync.dma_start(
                                t[:], xv[:, sqc * 8 * SQ:(sqc + 1) * 8 * SQ])
                            xts[sqc] = t

                        xv_dma(0)
                        xv_dma(1)
                        nc.sync.dma_start(xkr[:], xk[:, :])
                        nc.sync.dma_start(wsb["k"][:], wkd[:, :])
                        for sqc in range(nsqc):
                            if sqc + 2 < nsqc:
                                xv_dma(sqc + 2)
                            xt = xts.pop(sqc)
                            for st4 in range(SQ // 128):
                                i = sqc * (SQ // 128) + st4
                                ps = pp.tile([128, DC], f32, name="ps", tag="ps")
                                for k in range(8):
                                    nc.tensor.matmul(
                                        ps[:],
                                        xt[:, k * SQ + st4 * 128:
                                           k * SQ + (st4 + 1) * 128],
                                        wsb["v"][:, k * DC:(k + 1) * DC],
                                        start=(k == 0), stop=(k == 7))
                                src = ps[:].rearrange("p (h c) -> p h c", h=HPC)
                                dst = vt[i][:].rearrange(
                                    "p (h c) -> p h c", h=HPC)[:, :, 0:HD]
                                nc.vector.tensor_copy(dst, src)
                    nc.sync.dma_start(xqr[:], xq[:, :])
                    nc.sync.dma_start(wsb["q"][:], wqd[:, :])
                else:
                    for m in range(4):
                        nc.sync.dma_start(qt[m][:], xq[:, 0:seq_len])
                        nc.sync.dma_start(kt[m][:], xk[:, 0:seq_len])
                    for i in range(nskt):
                        dst = vt[i][:].rearrange(
                            "p (h c) -> p h c", h=HPC)[:, :, 0:HD]
                        off = (i % (seq_len // 512)) * 512
                        nc.sync.dma_start(
                            dst, xv[:, off:off + 512].rearrange(
                                "p (h c) -> p h c", h=HPC))

                # --- software-pipelined rounds ---------------------------
                # Round m runs attention for head-pair m.  The projection
                # matmuls for round m+1 (and, in the last round, the output
                # projection) are emitted two-at-a-time inside the skt loop,
                # filling the PE slots that would otherwise stall on exp.
                def proj_round_gen(m):
                    if "p" not in phases:
                        return
                    for name, xres, dstb in (("k", xkr, kt), ("q", xqr, qt)):
                        for sq0 in range(0, nsqc, 2):
                            cs = [c for c in (sq0, sq0 + 1) if c < nsqc]
                            pss = [pp.tile([128, SQ], f32, name="ps", tag="ps")
                                   for _ in cs]
                            for k in range(8):
                                for ci, sqc in enumerate(cs):
                                    nc.tensor.matmul(
                                        pss[ci][:],
                                        wsb[name][:, k * DC + m * 128:
                                                  k * DC + (m + 1) * 128],
                                        xres[:, k * seq_len + sqc * SQ:
                                             k * seq_len + (sqc + 1) * SQ],
                                        start=(k == 0), stop=(k == 7))
                                yield
                            for ci, sqc in enumerate(cs):
                                nc.vector.tensor_copy(
                                    dstb[m][:, sqc * SQ:(sqc + 1) * SQ],
                                    pss[ci][:])

                with tc.tile_pool(name="wo", bufs=1) as wop, \
                     tc.tile_pool(name="ob", bufs=2) as obp:
                    wo_sb = wop.tile([128, 4 * EMBED], bf16, name="wo",
                                     tag="wo")
                    if "o" in phases:
                        nc.sync.dma_start(wo_sb[:], wod[:, :])

                    def outproj_gen(mts):
                        if "o" not in phases:
                            return
                        for mt in mts:
                            ob = obp.tile([128, EMBED], f32, name="ob",
                                          tag="ob")
                            pss = [pp.tile([128, SQ], f32, name="ps", tag="ps")
                                   for _ in range(2)]
                            for kb in range(4):
                                for nch in range(2):
                                    nc.tensor.matmul(
                                        pss[nch][:],
                                        ot[kb][:, mt * 128:(mt + 1) * 128],
                                        wo_sb[:, kb * EMBED + nch * 512:
                                              kb * EMBED + (nch + 1) * 512],
                                        start=(kb == 0), stop=(kb == 3))
                                yield
                            for nch in range(2):
                                nc.vector.tensor_copy(
                                    ob[:, nch * 512:(nch + 1) * 512],
                                    pss[nch][:])
                            nc.sync.dma_start(
                                outd[mt * 128:(mt + 1) * 128, :], ob[:])

                    # round-0 projections up front
                    for _ in proj_round_gen(0):
                        pass

                    norm_group = {"den": rcp.tile([8, SQ], f32, name="den",
                                                  tag="den"),
                                  "rows": []}
                    for m in range(4):
                        # Rounds 0-2: filler = next round's projections;
                        # softmax norm flushed per 2 sq chunks (batched
                        # reciprocal).  Round 3: per-chunk norm flush; the
                        # out-projection chases it one chunk behind at 2
                        # filler matmul groups per slot.
                        gen = proj_round_gen(m + 1) if m < 3 else None
                        for sqc in range(nsqc):
                            if m == 3:
                                # out-proj of chunk sqc-1 (ot rows flushed)
                                gen = outproj_gen(
                                    range((sqc - 1) * 4, sqc * 4)) \
                                    if sqc > 0 and "a" in phases else None
                            if gen is None:
                                filler = lambda: None
                            elif m == 3:
                                filler = (lambda g=gen: (next(g, None),
                                                         next(g, None)))
                            else:
                                filler = (lambda g=gen: next(g, None))
                            if "a" in phases:
                                _attn_hp_sqc(nc, mybir, seq_len, m, sqc,
                                             qt, kt, vt, ot,
                                             ptp, rcp, stp, opp, filler,
                                             norm_group)
                                if m == 3 or sqc % 2 == 1:
                                    _flush_norm(nc, mybir, rcp, ot,
                                                norm_group)
                            else:
                                for _ in range(nskt):
                                    filler()
                            if m == 3 and gen is not None:
                                for _ in gen:  # drain this chunk's out-proj
                                    pass
                        if m < 3 and gen is not None:
                            for _ in gen:  # drain leftover filler work
                                pass
                        if "a" not in phases:
                            nc.sync.dma_start(ot[m][:], xq[:, 0:seq_len])
                    # tail: last sq chunk's out-projection
                    for _ in (outproj_gen(range(12, 16)) if "a" in phases
                              else outproj_gen(range(16))):
                        pass

    nc.compile()
    return nc


# ----------------------------------------------------------------- interface
def _prepare(inputs, seq_len, split_w=SPLIT_W):
    """Host-side fold + shard.  Returns (in_maps, bo)."""
    q_fold, q_beff = _quantum_fold(inputs["q_theta"], inputs["q_phi"],
                                   inputs["q_preW"], inputs["q_preb"],
                                   inputs["q_postW"], inputs["q_postb"])
    k_fold, k_beff = _quantum_fold(inputs["k_theta"], inputs["k_phi"],
                                   inputs["k_preW"], inputs["k_preb"],
                                   inputs["k_postW"], inputs["k_postb"])
    for b in (inputs["bq"], inputs["bk"], inputs["bv"], q_beff, k_beff):
        assert np.abs(np.asarray(b, dtype=np.float64)).max() == 0.0, \
            "nonzero bias path not implemented"

    scale_q = 1.0 / np.sqrt(np.float32(HD))
    wq_eff, wk_eff = _fold_weights(inputs["wq"], inputs["wk"],
                                   q_fold, k_fold, scale_q)
    wv_t = np.ascontiguousarray(inputs["wv"].T).astype(np.float32)
    wo = inputs["wo"]
    assert not split_w, "pre-tiled layouts assume unsplit weights"

    def tile_x(x):   # [E, S] -> [128, 8*S]:  [p, k*S+s] = x[128k+p, s]
        E, Sx = x.shape
        return np.ascontiguousarray(
            x.reshape(8, 128, Sx).transpose(1, 0, 2).reshape(128, 8 * Sx))

    def tile_xv(x):  # [E, S] -> [128, 4*8*512]: [p, c*4096+k*512+j]
        E, Sx = x.shape
        return np.ascontiguousarray(
            x.reshape(8, 128, Sx // SQ, SQ).transpose(1, 2, 0, 3)
            .reshape(128, 8 * Sx))

    def tile_w(w):   # [E, DC] -> [128, 8*DC]
        return np.ascontiguousarray(
            w.reshape(8, 128, DC).transpose(1, 0, 2).reshape(128, 8 * DC))

    B = inputs["query"].shape[0]
    xq_t = np.ascontiguousarray(inputs["query"].transpose(0, 2, 1)).astype(BF16)
    xk_t = np.ascontiguousarray(inputs["key"].transpose(0, 2, 1)).astype(BF16)
    xv_t = np.ascontiguousarray(inputs["value"].transpose(0, 2, 1)).astype(BF16)

    in_maps = []
    for c in range(N_CORES):
        b, hg = divmod(c, 2)
        b = b % B
        sl = slice(hg * DC, (hg + 1) * DC)
        wo_sl = np.ascontiguousarray(wo[:, sl].T).astype(BF16)  # [DC, E]
        in_maps.append({
            "xq_t": tile_x(xq_t[b, :, :seq_len]),
            "xk_t": tile_x(xk_t[b, :, :seq_len]),
            "xv_t": tile_xv(xv_t[b, :, :seq_len]),
            "wq_t": tile_w(wq_eff[:, sl].astype(BF16)),
            "wk_t": tile_w(wk_eff[:, sl].astype(BF16)),
            "wv_t": tile_w(wv_t[:, sl].astype(BF16)),
            "wo_t": np.ascontiguousarray(
                wo_sl.reshape(4, 128, EMBED).transpose(1, 0, 2)
                .reshape(128, 4 * EMBED)),
        })
    return in_maps, inputs["bo"]


def _install_hook_tracer():
    """Surface compile errors that the PJRT layer otherwise swallows."""
    if _CACHE.get("hook"):
        return
    _CACHE["hook"] = True
    try:
        from concourse import bass2jax
        bass2jax.install_neuronx_cc_hook()
        import libneuronxla
        orig = libneuronxla.neuronx_cc

        def wrapped(*args, **kwargs):
            try:
                return orig(*args, **kwargs)
            except Exception:
                import traceback
                traceback.print_exc()
                raise
        libneuronxla.neuronx_cc = wrapped
    except Exception:
        pass


def kernel(**inputs):
    from concourse.bass_utils import run_bass_kernel_spmd

    _install_hook_tracer()
    if "prog" not in _CACHE:
        _CACHE["prog"] = build_program(S)
    nc = _CACHE["prog"]

    in_maps, bo = _prepare(inputs, S)
    res = run_bass_kernel_spmd(nc, in_maps, core_ids=list(range(N_CORES)))

    B = inputs["query"].shape[0]
    out = np.empty((B, S, EMBED), dtype=np.float32)
    for b in range(B):
        out[b] = res.results[2 * b]["out"] + res.results[2 * b + 1]["out"]
    out += np.asarray(inputs["bo"]).reshape(1, 1, EMBED).astype(np.float32)
    return out

